# revision 34
# baseline (speedup 1.0000x reference)
"""Trainium2 Bass kernel for the quirky multi-head attention problem.

Math (per batch b, head a), faithful to the reference:
    K = x[b] @ W_K[a].T          # [S, H]
    Q = x[b] @ W_Q[a].T
    V = x[b] @ W_V[a].T
    s[c, C] = (K @ Q.T)[c, C] / sqrt(H)        rows c = "key" index
    valid iff C <= c (tril); softmax over C per row c
    E = exp(s) * tril            # no max-subtraction: |s| <= ~7, fp32-safe
    denom[c] = sum_C E[c, C]
    z[C, h] = sum_c E[c, C] * (V/denom)[c, h]  # = E.T @ (V/denom)
    out[b] += z @ W_O[a].T

Sharding: 8 cores = 2 batches x 4 head-pairs. Each core handles one batch
and two heads; the attention matrix is device-local. Host sums the four
head-pair partial outputs per batch.

PE-array tiling: score matmuls have K=64 (head dim) — two K=64 matmuls
at distinct row-groups (tile rows 0-63 / 64-127) stream CONCURRENTLY
through the systolic array. kt/qt are therefore stored per-head with the
head's values duplicated on both partition halves (stream_shuffle from
the projection wave); consecutive score chunks alternate row groups for
~2x score throughput. z matmuls have M=64 — the (j, j+4) chunk pair
targets PSUM partition halves 0/64 (distinct col-groups), emitted
back-to-back for concurrent streaming. Matmul N is capped at 512 (one
PSUM bank per matmul output).

v is computed transposed (v^T = W_V x^T, stationary shared across the
row) then flipped to [c, h] via PE transposes in 8-block groups.

Pipeline: scores [c_blk=128 rows, 1024-col waves of two 512 matmuls] go
to rotating PSUM wave tiles (2 bufs x 2 banks); the diagonal gets an
additive -1e9 triangle (GpSimd), ScalarE applies exp (scale=1/sqrt(H))
writing the fp16 row panel with fused per-row accumulation (softmax
denominator). z^T accumulates in PSUM across row blocks; chunk j of C
lives at partition half (j < NCH/2 ? 0 : 64) so z^T fits 4 banks; the
banks are pre-zeroed by K=1 dummy matmuls so every real z matmul uses
start=False (a start=True mid-stream would clear sibling bank state).
z matmuls lag Z_LAG blocks so PE streams without stalling on the softmax
chain. Head 1 sweeps rows in reverse so z banks finalize early; the
output projection is queued per-bank (ready at cb=4q) and paced one
unit per remaining block so ScalarE never starves on a cold epilogue.
k/q/v projection units are spread over head-0 blocks the same way.
"""

import math

import numpy as np

B, S_FULL, E, A, H = 2, 4096, 512, 8, 64
N_CORES = 8
NEG_BIG = -1.0e9

import os as _os

ATTN_DT = _os.environ.get("ATTN_DT", "fp16")
PROJ_DT = _os.environ.get("PROJ_DT", "fp16")
FILL_LDW = int(_os.environ.get("FILL_LDW", "0"))
Z_LAG = int(_os.environ.get("Z_LAG", "3"))
PANEL_BUFS = int(_os.environ.get("PANEL_BUFS", "4"))
GP_MASK = int(_os.environ.get("GP_MASK", "1"))

_prog_cache = {}


def _build_program(S, attn_dt=None, proj_dt=None):
    import concourse.mybir as mybir
    import concourse.tile as tile
    from concourse import bacc

    attn_dt = attn_dt or ATTN_DT
    proj_dt = proj_dt or PROJ_DT
    f32 = mybir.dt.float32
    f32r = mybir.dt.float32r
    bf16 = mybir.dt.bfloat16
    fp16 = mybir.dt.float16
    att_store = {"bf16": bf16, "fp16": fp16, "f32r": f32r, "f32": f32}[attn_dt]
    z_store = {"bf16": bf16, "fp16": fp16, "f32r": fp16, "f32": f32}[attn_dt]
    proj_store = {"f32r": f32r, "f32": f32, "fp16": fp16}[proj_dt]

    EC = E // 128            # e chunks (contraction for projections)
    NCB = S // 128           # row blocks
    NCH = S // 512           # C chunks per full row
    HALF = NCH // 2          # chunks per partition half of z^T
    NQ = 4                   # x quarters (kq/v projection units)
    QW = S // NQ
    assert NCH % 2 == 0 and QW == 1024

    nc = bacc.Bacc("TRN2", target_bir_lowering=False, debug=False)

    xT = nc.dram_tensor("xT", [E, S], proj_store, kind="ExternalInput")
    wk = nc.dram_tensor("wk", [E, 128], proj_store, kind="ExternalInput")
    wq = nc.dram_tensor("wq", [E, 128], proj_store, kind="ExternalInput")
    wv = nc.dram_tensor("wv", [E, 128], proj_store, kind="ExternalInput")
    wo0 = nc.dram_tensor("wo0", [128, E], proj_store, kind="ExternalInput")
    wo1 = nc.dram_tensor("wo1", [128, E], proj_store, kind="ExternalInput")
    maskb = nc.dram_tensor("maskb", [128, 128], f32, kind="ExternalInput")
    maskm = nc.dram_tensor("maskm", [128, 128], fp16, kind="ExternalInput")
    ident = nc.dram_tensor("ident", [128, 128], proj_store, kind="ExternalInput")
    outT = nc.dram_tensor("outT", [E, S], fp16, kind="ExternalOutput")

    ExpF = mybir.ActivationFunctionType.Exp
    AxX = mybir.AxisListType.X
    AluAdd = mybir.AluOpType.add

    with tile.TileContext(nc) as tc:
        with (
            tc.tile_pool(name="singles", bufs=1) as singles,
            tc.tile_pool(name="panelp", bufs=PANEL_BUFS) as panelp,
            tc.tile_pool(name="zsbp", bufs=2) as zsbp,
            tc.tile_pool(name="small", bufs=8) as small,
            tc.tile_pool(name="outst", bufs=4) as outst,
            tc.tile_pool(name="ps", bufs=2, space="PSUM") as ps,
            tc.tile_pool(name="zps", bufs=1, space="PSUM") as zps,
        ):
            # ---- load inputs ----
            # Critical path (xt q0, wk, wq, xt q1) issues on the sync queue
            # first; everything else goes through the gpsimd DGE so ~600ns
            # per-issue sync-queue serialization doesn't delay the first
            # score waves.
            xt = singles.tile([128, EC, S], proj_store)
            wks = singles.tile([128, EC, 128], proj_store)
            wqs = singles.tile([128, EC, 128], proj_store)
            wvs = singles.tile([128, EC, 128], proj_store)
            for ec in range(EC):
                nc.sync.dma_start(
                    out=xt[:, ec, 0:QW], in_=xT[ec * 128:(ec + 1) * 128, 0:QW]
                )
            for ec in range(EC):
                sl = slice(ec * 128, (ec + 1) * 128)
                nc.sync.dma_start(out=wks[:, ec, :], in_=wk[sl, :])
                nc.sync.dma_start(out=wqs[:, ec, :], in_=wq[sl, :])
                nc.sync.dma_start(out=wvs[:, ec, :], in_=wv[sl, :])
            for q in range(1, NQ):
                qsl = slice(q * QW, (q + 1) * QW)
                for ec in range(EC):
                    nc.sync.dma_start(
                        out=xt[:, ec, qsl], in_=xT[ec * 128:(ec + 1) * 128, qsl]
                    )
            wos0 = singles.tile([128, E], proj_store)
            wos1 = singles.tile([128, E], proj_store)
            nc.sync.dma_start(out=wos0, in_=wo0[:, :])
            nc.sync.dma_start(out=wos1, in_=wo1[:, :])
            msk = singles.tile([128, 128], f32)
            nc.sync.dma_start(out=msk, in_=maskb[:, :])
            mskm = singles.tile([128, 128], fp16)
            nc.sync.dma_start(out=mskm, in_=maskm[:, :])
            idt = singles.tile([128, 128], proj_store)
            nc.sync.dma_start(out=idt, in_=ident[:, :])
            # bf16 zeros: K=1 zeroing matmuls (invalid ISA in f32r); mixing
            # dtypes across a PSUM accumulation group is fine.
            zero_t = singles.tile([1, 576], bf16)
            nc.vector.memset(zero_t, 0.0)

            # ---- projection / transpose units (spread over head-0 blocks
            # so the PE feeds ScalarE instead of front-loading ACT-idle
            # projection lumps) ----
            # per-head kt/qt with the head duplicated on both partition
            # halves: consecutive score chunks alternate PE row-groups.
            ktd = (
                singles.tile([128, S], att_store, name="kt0"),
                singles.tile([128, S], att_store, name="kt1"),
            )
            qtd = (
                singles.tile([128, S], att_store, name="qt0"),
                singles.tile([128, S], att_store, name="qt1"),
            )
            vTs = singles.tile([128, S], fp16)       # v^T: rows 2 heads x 64
            vsb = singles.tile([128, NCB, 128], fp16)

            def emit_kq_cols(dsts, w, csl):
                cw = csl.stop - csl.start
                wt = ps.tile([128, 1024], f32, tag="wave", name="wt")
                for ec in range(EC):
                    for ci in range(cw // 512):
                        sub = slice(csl.start + ci * 512, csl.start + (ci + 1) * 512)
                        nc.tensor.matmul(
                            wt[:, ci * 512:(ci + 1) * 512], w[:, ec, :],
                            xt[:, ec, sub],
                            start=(ec == 0), stop=(ec == EC - 1),
                            skip_group_check=True,
                        )
                # aligned converting copies (DVE lanes can't cross
                # partitions), then SBUF->SBUF DMA duplicates across halves
                nc.vector.tensor_copy(dsts[0][0:64, csl], wt[0:64, :cw])
                nc.vector.tensor_copy(dsts[1][64:128, csl], wt[64:128, :cw])
                nc.sync.dma_start(out=dsts[0][64:128, csl], in_=dsts[0][0:64, csl])
                nc.sync.dma_start(out=dsts[1][0:64, csl], in_=dsts[1][64:128, csl])

            def emit_kq_unit(dsts, w, q):
                emit_kq_cols(dsts, w, slice(q * QW, (q + 1) * QW))

            def emit_vT_unit(q):
                wt = ps.tile([128, 1024], f32, tag="wave", name="wt")
                for ec in range(EC):
                    for ci in range(2):
                        csl = slice(q * QW + ci * 512, q * QW + (ci + 1) * 512)
                        nc.tensor.matmul(
                            wt[:, ci * 512:(ci + 1) * 512], wvs[:, ec, :],
                            xt[:, ec, csl],
                            start=(ec == 0), stop=(ec == EC - 1),
                            skip_group_check=True,
                        )
                nc.vector.tensor_copy(vTs[:, q * QW:(q + 1) * QW], wt[:, :QW])

            def emit_vtr_unit(g):
                # flip 8 blocks of v^T into vsb[:, 8g:8g+8, :] via PE
                # transposes (fp16 PSUM tile, same byte size as a wave tile)
                tt = ps.tile([128, 2048], fp16, tag="wave", name="tt")
                for k in range(8):
                    cb = 8 * g + k
                    nc.tensor.matmul(
                        tt[:, k * 128:(k + 1) * 128],
                        vTs[:, cb * 128:(cb + 1) * 128], idt,
                        is_transpose=True,
                    )
                nc.vector.tensor_copy(vsb[:, 8 * g:8 * (g + 1), :], tt[:, :1024])

            wos = (wos0, wos1)

            # ---- attention over both heads, sequential; per-head z^T in
            # 4 PSUM banks; unit queue paces projection/out-proj work ----
            zsb_heads = []
            for h in range(2):
                # head 0 forward, head 1 reverse: the reverse head leads with
                # its widest blocks; z banks finalize early so the output
                # projection trickles into the narrow-block tail.
                order = list(range(NCB - 1, -1, -1)) if h == 1 else list(range(NCB))
                zT = zps.tile([128, HALF * 512], f32, name="zT")
                # Zero all banks/halves with dummy K=1 matmuls; every real z
                # matmul then accumulates with start=False (a start=True
                # mid-stream would clear sibling bank state).
                for poff in (0, 64):
                    for bq in range(HALF):
                        nc.tensor.matmul(
                            zT[poff:poff + 64, bq * 512:(bq + 1) * 512],
                            zero_t[:, :64], zero_t[:, 64:576],
                            start=True, stop=False, skip_group_check=True,
                        )

                zsb = zsbp.tile([128, HALF * 512], proj_store, name="zsb")
                zsb_heads.append(zsb)

                last_lo = order[-1]
                last_hi = (4 * HALF) if h == 1 else (NCB - 1)

                def emit_out_unit(q, ecn, flush=False):
                    # chunk q (poff 0) + chunk q+HALF (poff 64) share zsb
                    # cols; one [128,1024] wave, two 512 regions, 2 heads.
                    # Flush-phase units (after the last exp) route the copy
                    # to the now-idle scalar engine and DMA issues to the
                    # gpsimd DGE so DVE/sync don't serialize the tail.
                    col = q * 512
                    esl = slice(ecn * 128, (ecn + 1) * 128)
                    wt = ps.tile([128, 1024], f32, tag="wave", name="wt")
                    for ci, poff in ((0, 0), (1, 64)):
                        for hh in range(2):
                            nc.tensor.matmul(
                                wt[:, ci * 512:(ci + 1) * 512],
                                wos[hh][poff:poff + 64, esl],
                                zsb_heads[hh][poff:poff + 64, col:col + 512],
                                start=(hh == 0), stop=(hh == 1),
                                skip_group_check=True,
                            )
                    st = outst.tile([128, 1024], fp16, name="st")
                    if flush:
                        # after the last exp ScalarE is idle; routing the
                        # PSUM->SBUF copy there unblocks the wave pool while
                        # DVE drains the other tail copies
                        nc.scalar.copy(st, wt[:, :1024])
                    else:
                        nc.vector.tensor_copy(st, wt[:, :1024])
                    nc.sync.dma_start(
                        out=outT[esl, q * 512:(q + 1) * 512], in_=st[:, :512]
                    )
                    nc.sync.dma_start(
                        out=outT[esl, (q + HALF) * 512:(q + HALF + 1) * 512],
                        in_=st[:, 512:1024],
                    )

                def emit_z(item):
                    vt_i, panel_i, nch_i, cb_i = item
                    lo = min(nch_i, HALF)
                    hi = nch_i - lo
                    # sequential same-col-group emission: concurrent drains
                    # into one PSUM bank (partition halves) backpressure the
                    # PSUM crossbar and slow ACT reads chip-wide
                    for j in range(lo):
                        nc.tensor.matmul(
                            zT[0:64, j * 512:(j + 1) * 512], vt_i,
                            panel_i[:, j * 512:(j + 1) * 512],
                            start=False, stop=(cb_i == last_lo),
                            skip_group_check=True,
                        )
                    for j in range(hi):
                        nc.tensor.matmul(
                            zT[64:128, j * 512:(j + 1) * 512], vt_i,
                            panel_i[:, (j + HALF) * 512:(j + HALF + 1) * 512],
                            start=False, stop=(cb_i == last_hi),
                            skip_group_check=True,
                        )
                    # Head 1 reverse: bank q (chunks q, q+HALF) is final once
                    # cb=4q is done; queue its zsb copy + output projection,
                    # paced one unit per block so ACT never starves.
                    if h == 1 and cb_i % 4 == 0 and cb_i // 4 < HALF:
                        q = cb_i // 4
                        nc.vector.tensor_copy(
                            zsb[:, q * 512:(q + 1) * 512],
                            zT[:, q * 512:(q + 1) * 512],
                        )
                        for ecn in range(EC):
                            unit_queue.append(
                                lambda q=q, ecn=ecn, **kw: emit_out_unit(q, ecn, **kw)
                            )

                # head-0 unit schedule (emitted after block oi's score waves).
                # Deadlines: kq quarter q' before block 8q', vtr group g
                # before block 8g (vt lags Z_LAG so group 0 has slack).
                unit_queue = []
                if h == 0:
                    emit_kq_unit(ktd, wks, 0)
                    emit_kq_unit(qtd, wqs, 0)
                    sched = {
                        0: [lambda: emit_vT_unit(0), lambda: emit_vtr_unit(0)],
                        1: [lambda: emit_vT_unit(1)],
                        2: [lambda: emit_kq_unit(ktd, wks, 1)],
                        3: [lambda: emit_kq_unit(qtd, wqs, 1)],
                        4: [lambda: emit_vtr_unit(1)],
                        5: [lambda: emit_vT_unit(2)],
                        6: [lambda: emit_kq_unit(ktd, wks, 2)],
                        7: [lambda: emit_kq_unit(qtd, wqs, 2)],
                        10: [lambda: emit_vtr_unit(2)],
                        11: [lambda: emit_vT_unit(3)],
                        12: [lambda: emit_kq_unit(ktd, wks, 3)],
                        13: [lambda: emit_kq_unit(qtd, wqs, 3)],
                        18: [lambda: emit_vtr_unit(3)],
                    }
                else:
                    sched = {}

                pending = []
                for oi, cb in enumerate(order):
                    c0 = cb * 128
                    nch = (c0 + 128 + 511) // 512
                    nwaves = (nch + 1) // 2
                    lastw = c0 + 128 - (nch - 1) * 512   # width of diag chunk
                    panel = panelp.tile([128, S], z_store, name="panel")
                    if lastw < 512:
                        # zero the diag chunk tail so z matmuls read zeros
                        nc.gpsimd.memset(
                            panel[:, (nch - 1) * 512 + lastw:nch * 512], 0.0
                        )
                    rsp = small.tile([128, 4], f32, name="rsp")
                    for wv_i in range(nwaves):
                        jlo = 2 * wv_i
                        jhi = min(jlo + 2, nch)
                        wt = ps.tile([128, 1024], f32, tag="wave", name="wt")
                        for j in range(jlo, jhi):
                            w_n = lastw if j == nch - 1 else 512
                            rg = 64 * (j % 2)
                            nc.tensor.matmul(
                                wt[:, (j - jlo) * 512:(j - jlo) * 512 + w_n],
                                ktd[h][rg:rg + 64, c0:c0 + 128],
                                qtd[h][rg:rg + 64, j * 512:j * 512 + w_n],
                                start=True, stop=True,
                            )
                        if jhi == nch and nwaves > 1:
                            # mask only the last 128 cols (the true triangle);
                            # earlier diag-chunk cols are fully valid
                            o = c0 - (nch - 1) * 512
                            dlo = (nch - 1 - jlo) * 512 + o
                            # PSUM op: must be DVE (walrus forbids gpsimd-PSUM)
                            nc.vector.tensor_add(
                                wt[:, dlo:dlo + 128], wt[:, dlo:dlo + 128],
                                msk,
                            )
                        nc.scalar.activation(
                            out=panel[:, jlo * 512:jlo * 512 + wlen_of(jhi, jlo, nch, lastw)],
                            in_=wt[:, :wlen_of(jhi, jlo, nch, lastw)],
                            func=ExpF,
                            scale=1.0 / math.sqrt(H),
                            # single-wave blocks: skip the ~287ns ACT
                            # accumulator read; DVE reduces the fp16 panel
                            accum_out=None if nwaves == 1 else rsp[:, wv_i:wv_i + 1],
                        )
                    if h == 0 and oi in sched:
                        for fn in sched[oi]:
                            fn()
                    if unit_queue:
                        unit_queue.pop(0)()
                    den = small.tile([128, 1], f32, name="den")
                    if nwaves > 1:
                        nc.vector.tensor_reduce(den, rsp[:, :nwaves], axis=AxX, op=AluAdd)
                    else:
                        # single-wave: mask applied post-exp (0/1 triangle on
                        # the fp16 panel) so exp never waits the mask; the
                        # reduce then sums the masked zeros
                        dpan = (nch - 1) * 512 + (c0 - (nch - 1) * 512)
                        nc.gpsimd.tensor_mul(
                            panel[:, dpan:dpan + 128],
                            panel[:, dpan:dpan + 128], mskm,
                        )
                        nc.vector.tensor_reduce(
                            den, panel[:, :nch * 512], axis=AxX, op=AluAdd
                        )
                    rden = small.tile([128, 1], f32, name="rden")
                    nc.vector.reciprocal(rden, den)
                    vt = small.tile([128, 64], z_store, name="vt")
                    nc.vector.tensor_scalar_mul(vt, vsb[:, cb, hs_of(h)], rden)
                    pending.append((vt, panel, nch, cb))
                    if len(pending) > Z_LAG:
                        emit_z(pending.pop(0))
                    # dependency-free weight loads keep the PE activity monitor
                    # from re-throttling the clock during ACT-gated idles
                    for _ in range(FILL_LDW):
                        nc.tensor.ldweights(zero_t[:, :128])
                for item in pending:
                    emit_z(item)
                while unit_queue:
                    unit_queue.pop(0)(flush=True)
                if h == 0:
                    for bq in range(HALF):
                        nc.vector.tensor_copy(
                            zsb[:, bq * 512:(bq + 1) * 512],
                            zT[:, bq * 512:(bq + 1) * 512],
                        )

    nc.compile()
    return nc


def wlen_of(jhi, jlo, nch, lastw):
    return (jhi - jlo - 1) * 512 + (lastw if jhi == nch else 512)


def hs_of(h):
    return slice(h * 64, (h + 1) * 64)


def get_program(S=S_FULL):
    if S not in _prog_cache:
        _prog_cache[S] = _build_program(S)
    return _prog_cache[S]


def make_mask_band():
    """Triangle mask for the last 128 cols of a diagonal chunk:
    col t (relative to the diagonal start) is valid iff t <= r."""
    r = np.arange(128)[:, None]
    t = np.arange(128)[None, :]
    return np.where(t <= r, 0.0, NEG_BIG).astype(np.float32)


def make_core_inputs(x, W_K, W_Q, W_V, W_O, core):
    """Inputs for core = b*4 + g (batch b, head pair a0=2g, a1=2g+1)."""
    b, g = divmod(core, 4)
    a0, a1 = 2 * g, 2 * g + 1
    pdt = np.float16 if PROJ_DT == "fp16" else np.float32
    xT = np.ascontiguousarray(x[b].T).astype(pdt)
    wk = np.ascontiguousarray(np.concatenate([W_K[a0].T, W_K[a1].T], axis=1)).astype(pdt)
    wq = np.ascontiguousarray(np.concatenate([W_Q[a0].T, W_Q[a1].T], axis=1)).astype(pdt)
    wv = np.ascontiguousarray(np.concatenate([W_V[a0].T, W_V[a1].T], axis=1)).astype(pdt)
    wo0 = np.ascontiguousarray(np.concatenate([W_O[a0].T, W_O[a0].T], axis=0)).astype(pdt)
    wo1 = np.ascontiguousarray(np.concatenate([W_O[a1].T, W_O[a1].T], axis=0)).astype(pdt)
    tri = make_mask_band()
    return {
        "xT": xT, "wk": wk, "wq": wq, "wv": wv,
        "wo0": wo0, "wo1": wo1, "maskb": tri,
        "maskm": (tri == 0.0).astype(np.float16),
        "ident": np.eye(128, dtype=pdt),
    }


def run_on_cores(inputs, trace=False):
    from concourse.bass_utils import run_bass_kernel_spmd

    nc = get_program()
    in_maps = [
        make_core_inputs(
            inputs["x"], inputs["W_K"], inputs["W_Q"], inputs["W_V"],
            inputs["W_O"], core,
        )
        for core in range(N_CORES)
    ]
    return run_bass_kernel_spmd(
        nc, in_maps, list(range(N_CORES)), trace=trace,
    )


def kernel(x, W_K, W_Q, W_V, W_O):
    x = np.asarray(x, dtype=np.float32)
    W_K = np.asarray(W_K, dtype=np.float32)
    W_Q = np.asarray(W_Q, dtype=np.float32)
    W_V = np.asarray(W_V, dtype=np.float32)
    W_O = np.asarray(W_O, dtype=np.float32)
    res = run_on_cores(
        {"x": x, "W_K": W_K, "W_Q": W_Q, "W_V": W_V, "W_O": W_O}
    )
    out = np.zeros((B, S_FULL, E), dtype=np.float32)
    for b in range(B):
        acc = np.zeros((E, S_FULL), dtype=np.float32)
        for g in range(4):
            acc += res.results[b * 4 + g]["outT"].astype(np.float32)
        out[b] = acc.T
    return out


# revision 37
# speedup vs baseline: 1.0081x; 1.0081x over previous
"""Trainium2 Bass kernel for the quirky multi-head attention problem.

Math (per batch b, head a), faithful to the reference:
    K = x[b] @ W_K[a].T          # [S, H]
    Q = x[b] @ W_Q[a].T
    V = x[b] @ W_V[a].T
    s[c, C] = (K @ Q.T)[c, C] / sqrt(H)        rows c = "key" index
    valid iff C <= c (tril); softmax over C per row c
    E = exp(s) * tril            # no max-subtraction: |s| <= ~7, fp32-safe
    denom[c] = sum_C E[c, C]
    z[C, h] = sum_c E[c, C] * (V/denom)[c, h]  # = E.T @ (V/denom)
    out[b] += z @ W_O[a].T

Sharding: 8 cores = 2 batches x 4 head-pairs. Each core handles one batch
and two heads; the attention matrix is device-local. Host sums the four
head-pair partial outputs per batch.

PE-array tiling: score matmuls have K=64 (head dim) — two K=64 matmuls
at distinct row-groups (tile rows 0-63 / 64-127) stream CONCURRENTLY
through the systolic array (~2x score throughput; measured 4-7ns start
deltas). kt/qt are therefore stored per-head with the head's values
duplicated on both partition halves (DVE aligned copy from the wave,
then an SBUF->SBUF DMA for the crossed half — DVE lanes and
stream_shuffle cannot cross the 64-partition boundary); consecutive
score chunks alternate row groups. Matmul N is capped at 512 (one PSUM
bank per matmul output; wider fails the s3d3 ISA check). z chunk halves
are emitted as separate sequential loops: interleaving the (j, j+4)
pair (same PSUM bank, partition halves 0/64) made every ACT/DVE
instruction ~20% slower chip-wide.

v is computed transposed (v^T = W_V x^T, stationary shared across the
row) then flipped to [c, h] via PE transposes in 8-block groups.

Pipeline: scores [c_blk=128 rows, 1024-col waves of two 512 matmuls] go
to rotating PSUM wave tiles (2 bufs x 2 banks); the diagonal gets an
additive -1e9 triangle (DVE; gpsimd cannot touch PSUM), ScalarE applies
exp (scale=1/sqrt(H)) writing the fp16 row panel with fused per-row
accumulation (softmax denominator). z^T accumulates in PSUM across row
blocks; chunk j of C lives at partition half (j < NCH/2 ? 0 : 64) so
z^T fits 4 banks; the banks are pre-zeroed by K=1 dummy matmuls so
every real z matmul uses start=False (a start=True mid-stream would
clear sibling bank state). z matmuls lag Z_LAG blocks so PE streams
without stalling on the softmax chain. Head 1 sweeps rows in reverse so
z banks finalize early; the output projection is queued per-bank (ready
at cb=4q) and paced one unit per remaining block so ScalarE never
starves on a cold epilogue. k/q/v projection units are spread over
head-0 blocks the same way. ScalarE (exp payload + accumulator reads,
~213us busy) is the bottleneck engine; PE sits at ~190us busy.
"""

import math

import numpy as np

B, S_FULL, E, A, H = 2, 4096, 512, 8, 64
N_CORES = 8
NEG_BIG = -1.0e9

import os as _os

ATTN_DT = _os.environ.get("ATTN_DT", "fp16")
PROJ_DT = _os.environ.get("PROJ_DT", "fp16")
FILL_LDW = int(_os.environ.get("FILL_LDW", "0"))
Z_LAG = int(_os.environ.get("Z_LAG", "3"))
PANEL_BUFS = int(_os.environ.get("PANEL_BUFS", "4"))

_prog_cache = {}


def _build_program(S, attn_dt=None, proj_dt=None):
    import concourse.mybir as mybir
    import concourse.tile as tile
    from concourse import bacc

    attn_dt = attn_dt or ATTN_DT
    proj_dt = proj_dt or PROJ_DT
    f32 = mybir.dt.float32
    f32r = mybir.dt.float32r
    bf16 = mybir.dt.bfloat16
    fp16 = mybir.dt.float16
    att_store = {"bf16": bf16, "fp16": fp16, "f32r": f32r, "f32": f32}[attn_dt]
    z_store = {"bf16": bf16, "fp16": fp16, "f32r": fp16, "f32": f32}[attn_dt]
    proj_store = {"f32r": f32r, "f32": f32, "fp16": fp16}[proj_dt]

    EC = E // 128            # e chunks (contraction for projections)
    NCB = S // 128           # row blocks
    NCH = S // 512           # C chunks per full row
    HALF = NCH // 2          # chunks per partition half of z^T
    NQ = 4                   # x quarters (kq/v projection units)
    QW = S // NQ
    assert NCH % 2 == 0 and QW == 1024

    nc = bacc.Bacc("TRN2", target_bir_lowering=False, debug=False)

    xT = nc.dram_tensor("xT", [E, S], proj_store, kind="ExternalInput")
    wk = nc.dram_tensor("wk", [E, 128], proj_store, kind="ExternalInput")
    wq = nc.dram_tensor("wq", [E, 128], proj_store, kind="ExternalInput")
    wv = nc.dram_tensor("wv", [E, 128], proj_store, kind="ExternalInput")
    wo0 = nc.dram_tensor("wo0", [128, E], proj_store, kind="ExternalInput")
    wo1 = nc.dram_tensor("wo1", [128, E], proj_store, kind="ExternalInput")
    maskb = nc.dram_tensor("maskb", [128, 128], f32, kind="ExternalInput")
    maskm = nc.dram_tensor("maskm", [128, 128], fp16, kind="ExternalInput")
    ident = nc.dram_tensor("ident", [128, 128], proj_store, kind="ExternalInput")
    outT = nc.dram_tensor("outT", [E, S], fp16, kind="ExternalOutput")

    ExpF = mybir.ActivationFunctionType.Exp
    AxX = mybir.AxisListType.X
    AluAdd = mybir.AluOpType.add

    with tile.TileContext(nc) as tc:
        with (
            tc.tile_pool(name="singles", bufs=1) as singles,
            tc.tile_pool(name="panelp", bufs=PANEL_BUFS) as panelp,
            tc.tile_pool(name="zsbp", bufs=2) as zsbp,
            tc.tile_pool(name="small", bufs=8) as small,
            tc.tile_pool(name="outst", bufs=4) as outst,
            tc.tile_pool(name="ps", bufs=2, space="PSUM") as ps,
            tc.tile_pool(name="zps", bufs=1, space="PSUM") as zps,
        ):
            # ---- load inputs ----
            # Critical path (xt q0, wk, wq, xt q1) issues on the sync queue
            # first; everything else goes through the gpsimd DGE so ~600ns
            # per-issue sync-queue serialization doesn't delay the first
            # score waves.
            xt = singles.tile([128, EC, S], proj_store)
            wks = singles.tile([128, EC, 128], proj_store)
            wqs = singles.tile([128, EC, 128], proj_store)
            wvs = singles.tile([128, EC, 128], proj_store)
            for ec in range(EC):
                nc.sync.dma_start(
                    out=xt[:, ec, 0:QW], in_=xT[ec * 128:(ec + 1) * 128, 0:QW]
                )
            for ec in range(EC):
                sl = slice(ec * 128, (ec + 1) * 128)
                nc.sync.dma_start(out=wks[:, ec, :], in_=wk[sl, :])
                nc.sync.dma_start(out=wqs[:, ec, :], in_=wq[sl, :])
                nc.sync.dma_start(out=wvs[:, ec, :], in_=wv[sl, :])
            for q in range(1, NQ):
                qsl = slice(q * QW, (q + 1) * QW)
                for ec in range(EC):
                    nc.sync.dma_start(
                        out=xt[:, ec, qsl], in_=xT[ec * 128:(ec + 1) * 128, qsl]
                    )
            wos0 = singles.tile([128, E], proj_store)
            wos1 = singles.tile([128, E], proj_store)
            nc.sync.dma_start(out=wos0, in_=wo0[:, :])
            nc.sync.dma_start(out=wos1, in_=wo1[:, :])
            msk = singles.tile([128, 128], f32)
            nc.sync.dma_start(out=msk, in_=maskb[:, :])
            mskm = singles.tile([128, 128], fp16)
            nc.sync.dma_start(out=mskm, in_=maskm[:, :])
            idt = singles.tile([128, 128], proj_store)
            nc.sync.dma_start(out=idt, in_=ident[:, :])
            # bf16 zeros: K=1 zeroing matmuls (invalid ISA in f32r); mixing
            # dtypes across a PSUM accumulation group is fine.
            zero_t = singles.tile([1, 576], bf16)
            nc.vector.memset(zero_t, 0.0)

            # ---- projection / transpose units (spread over head-0 blocks
            # so the PE feeds ScalarE instead of front-loading ACT-idle
            # projection lumps) ----
            # per-head kt/qt with the head duplicated on both partition
            # halves: consecutive score chunks alternate PE row-groups.
            ktd = (
                singles.tile([128, S], att_store, name="kt0"),
                singles.tile([128, S], att_store, name="kt1"),
            )
            qtd = (
                singles.tile([128, S], att_store, name="qt0"),
                singles.tile([128, S], att_store, name="qt1"),
            )
            vTs = singles.tile([128, S], fp16)       # v^T: rows 2 heads x 64
            vsb = singles.tile([128, NCB, 128], fp16)

            def emit_kq_cols(dsts, w, csl):
                cw = csl.stop - csl.start
                wt = ps.tile([128, 1024], f32, tag="wave", name="wt")
                for ec in range(EC):
                    for ci in range(cw // 512):
                        sub = slice(csl.start + ci * 512, csl.start + (ci + 1) * 512)
                        nc.tensor.matmul(
                            wt[:, ci * 512:(ci + 1) * 512], w[:, ec, :],
                            xt[:, ec, sub],
                            start=(ec == 0), stop=(ec == EC - 1),
                            skip_group_check=True,
                        )
                # aligned converting copies (DVE lanes can't cross
                # partitions), then SBUF->SBUF DMA duplicates across halves
                nc.vector.tensor_copy(dsts[0][0:64, csl], wt[0:64, :cw])
                nc.vector.tensor_copy(dsts[1][64:128, csl], wt[64:128, :cw])
                nc.sync.dma_start(out=dsts[0][64:128, csl], in_=dsts[0][0:64, csl])
                nc.sync.dma_start(out=dsts[1][0:64, csl], in_=dsts[1][64:128, csl])

            def emit_kq_unit(dsts, w, q):
                emit_kq_cols(dsts, w, slice(q * QW, (q + 1) * QW))

            def emit_vT_unit(q):
                wt = ps.tile([128, 1024], f32, tag="wave", name="wt")
                for ec in range(EC):
                    for ci in range(2):
                        csl = slice(q * QW + ci * 512, q * QW + (ci + 1) * 512)
                        nc.tensor.matmul(
                            wt[:, ci * 512:(ci + 1) * 512], wvs[:, ec, :],
                            xt[:, ec, csl],
                            start=(ec == 0), stop=(ec == EC - 1),
                            skip_group_check=True,
                        )
                nc.vector.tensor_copy(vTs[:, q * QW:(q + 1) * QW], wt[:, :QW])

            def emit_vtr_unit(g):
                # flip 8 blocks of v^T into vsb[:, 8g:8g+8, :] via PE
                # transposes (fp16 PSUM tile, same byte size as a wave tile)
                tt = ps.tile([128, 2048], fp16, tag="wave", name="tt")
                for k in range(8):
                    cb = 8 * g + k
                    nc.tensor.matmul(
                        tt[:, k * 128:(k + 1) * 128],
                        vTs[:, cb * 128:(cb + 1) * 128], idt,
                        is_transpose=True,
                    )
                nc.vector.tensor_copy(vsb[:, 8 * g:8 * (g + 1), :], tt[:, :1024])

            wos = (wos0, wos1)

            # ---- attention over both heads, sequential; per-head z^T in
            # 4 PSUM banks; unit queue paces projection/out-proj work ----
            zsb_heads = []
            for h in range(2):
                # head 0 forward, head 1 reverse: the reverse head leads with
                # its widest blocks; z banks finalize early so the output
                # projection trickles into the narrow-block tail.
                order = list(range(NCB - 1, -1, -1)) if h == 1 else list(range(NCB))
                zT = zps.tile([128, HALF * 512], f32, name="zT")
                # Zero all banks/halves with dummy K=1 matmuls; every real z
                # matmul then accumulates with start=False (a start=True
                # mid-stream would clear sibling bank state).
                for poff in (0, 64):
                    for bq in range(HALF):
                        nc.tensor.matmul(
                            zT[poff:poff + 64, bq * 512:(bq + 1) * 512],
                            zero_t[:, :64], zero_t[:, 64:576],
                            start=True, stop=False, skip_group_check=True,
                        )

                zsb = zsbp.tile([128, HALF * 512], proj_store, name="zsb")
                zsb_heads.append(zsb)

                last_lo = order[-1]
                last_hi = (4 * HALF) if h == 1 else (NCB - 1)

                def emit_out_unit(q, ecn, flush=False):
                    # chunk q (poff 0) + chunk q+HALF (poff 64) share zsb
                    # cols; one [128,1024] wave, two 512 regions, 2 heads.
                    # Flush-phase units (after the last exp) route the copy
                    # to the now-idle scalar engine and DMA issues to the
                    # gpsimd DGE so DVE/sync don't serialize the tail.
                    col = q * 512
                    esl = slice(ecn * 128, (ecn + 1) * 128)
                    wt = ps.tile([128, 1024], f32, tag="wave", name="wt")
                    for ci, poff in ((0, 0), (1, 64)):
                        for hh in range(2):
                            nc.tensor.matmul(
                                wt[:, ci * 512:(ci + 1) * 512],
                                wos[hh][poff:poff + 64, esl],
                                zsb_heads[hh][poff:poff + 64, col:col + 512],
                                start=(hh == 0), stop=(hh == 1),
                                skip_group_check=True,
                            )
                    st = outst.tile([128, 1024], fp16, name="st")
                    nc.vector.tensor_copy(st, wt[:, :1024])
                    nc.sync.dma_start(
                        out=outT[esl, q * 512:(q + 1) * 512], in_=st[:, :512]
                    )
                    nc.sync.dma_start(
                        out=outT[esl, (q + HALF) * 512:(q + HALF + 1) * 512],
                        in_=st[:, 512:1024],
                    )

                def emit_z(item):
                    vt_i, panel_i, nch_i, cb_i = item
                    lo = min(nch_i, HALF)
                    hi = nch_i - lo
                    # sequential same-col-group emission: concurrent drains
                    # into one PSUM bank (partition halves) backpressure the
                    # PSUM crossbar and slow ACT reads chip-wide
                    for j in range(lo):
                        nc.tensor.matmul(
                            zT[0:64, j * 512:(j + 1) * 512], vt_i,
                            panel_i[:, j * 512:(j + 1) * 512],
                            start=False, stop=(cb_i == last_lo),
                            skip_group_check=True,
                        )
                    for j in range(hi):
                        nc.tensor.matmul(
                            zT[64:128, j * 512:(j + 1) * 512], vt_i,
                            panel_i[:, (j + HALF) * 512:(j + HALF + 1) * 512],
                            start=False, stop=(cb_i == last_hi),
                            skip_group_check=True,
                        )
                    # Head 1 reverse: bank q (chunks q, q+HALF) is final once
                    # cb=4q is done; queue its zsb copy + output projection,
                    # paced one unit per block so ACT never starves.
                    if h == 1 and cb_i % 4 == 0 and cb_i // 4 < HALF:
                        q = cb_i // 4
                        nc.vector.tensor_copy(
                            zsb[:, q * 512:(q + 1) * 512],
                            zT[:, q * 512:(q + 1) * 512],
                        )
                        for ecn in range(EC):
                            unit_queue.append(
                                lambda q=q, ecn=ecn, **kw: emit_out_unit(q, ecn, **kw)
                            )

                # head-0 unit schedule (emitted after block oi's score waves).
                # Deadlines: kq quarter q' before block 8q', vtr group g
                # before block 8g (vt lags Z_LAG so group 0 has slack).
                unit_queue = []
                if h == 0:
                    emit_kq_unit(ktd, wks, 0)
                    emit_kq_unit(qtd, wqs, 0)
                    sched = {
                        0: [lambda: emit_vT_unit(0), lambda: emit_vtr_unit(0)],
                        1: [lambda: emit_vT_unit(1)],
                        2: [lambda: emit_kq_unit(ktd, wks, 1)],
                        3: [lambda: emit_kq_unit(qtd, wqs, 1)],
                        4: [lambda: emit_vtr_unit(1)],
                        5: [lambda: emit_vT_unit(2)],
                        6: [lambda: emit_kq_unit(ktd, wks, 2)],
                        7: [lambda: emit_kq_unit(qtd, wqs, 2)],
                        10: [lambda: emit_vtr_unit(2)],
                        11: [lambda: emit_vT_unit(3)],
                        12: [lambda: emit_kq_unit(ktd, wks, 3)],
                        13: [lambda: emit_kq_unit(qtd, wqs, 3)],
                        18: [lambda: emit_vtr_unit(3)],
                    }
                else:
                    sched = {}

                pending = []
                for oi, cb in enumerate(order):
                    c0 = cb * 128
                    nch = (c0 + 128 + 511) // 512
                    nwaves = (nch + 1) // 2
                    lastw = c0 + 128 - (nch - 1) * 512   # width of diag chunk
                    panel = panelp.tile([128, S], z_store, name="panel")
                    if lastw < 512:
                        # zero the diag chunk tail so z matmuls read zeros
                        nc.gpsimd.memset(
                            panel[:, (nch - 1) * 512 + lastw:nch * 512], 0.0
                        )
                    rsp = small.tile([128, 4], f32, name="rsp")
                    for wv_i in range(nwaves):
                        jlo = 2 * wv_i
                        jhi = min(jlo + 2, nch)
                        wt = ps.tile([128, 1024], f32, tag="wave", name="wt")
                        for j in range(jlo, jhi):
                            w_n = lastw if j == nch - 1 else 512
                            rg = 64 * (j % 2)
                            nc.tensor.matmul(
                                wt[:, (j - jlo) * 512:(j - jlo) * 512 + w_n],
                                ktd[h][rg:rg + 64, c0:c0 + 128],
                                qtd[h][rg:rg + 64, j * 512:j * 512 + w_n],
                                start=True, stop=True,
                            )
                        if jhi == nch and nwaves > 1:
                            # mask only the last 128 cols (the true triangle);
                            # earlier diag-chunk cols are fully valid
                            o = c0 - (nch - 1) * 512
                            dlo = (nch - 1 - jlo) * 512 + o
                            # PSUM op: must be DVE (walrus forbids gpsimd-PSUM)
                            nc.vector.tensor_add(
                                wt[:, dlo:dlo + 128], wt[:, dlo:dlo + 128],
                                msk,
                            )
                        nc.scalar.activation(
                            out=panel[:, jlo * 512:jlo * 512 + wlen_of(jhi, jlo, nch, lastw)],
                            in_=wt[:, :wlen_of(jhi, jlo, nch, lastw)],
                            func=ExpF,
                            scale=1.0 / math.sqrt(H),
                            # single-wave blocks: skip the ~287ns ACT
                            # accumulator read; DVE reduces the fp16 panel
                            accum_out=None if nwaves == 1 else rsp[:, wv_i:wv_i + 1],
                        )
                    if h == 0 and oi in sched:
                        for fn in sched[oi]:
                            fn()
                    if unit_queue:
                        unit_queue.pop(0)()
                    den = small.tile([128, 1], f32, name="den")
                    if nwaves > 1:
                        nc.vector.tensor_reduce(den, rsp[:, :nwaves], axis=AxX, op=AluAdd)
                    else:
                        # single-wave: mask applied post-exp (0/1 triangle on
                        # the fp16 panel) so exp never waits the mask; the
                        # reduce then sums the masked zeros
                        dpan = (nch - 1) * 512 + (c0 - (nch - 1) * 512)
                        nc.gpsimd.tensor_mul(
                            panel[:, dpan:dpan + 128],
                            panel[:, dpan:dpan + 128], mskm,
                        )
                        nc.vector.tensor_reduce(
                            den, panel[:, :nch * 512], axis=AxX, op=AluAdd
                        )
                    rden = small.tile([128, 1], f32, name="rden")
                    nc.vector.reciprocal(rden, den)
                    vt = small.tile([128, 64], z_store, name="vt")
                    nc.vector.tensor_scalar_mul(vt, vsb[:, cb, hs_of(h)], rden)
                    pending.append((vt, panel, nch, cb))
                    if len(pending) > Z_LAG:
                        emit_z(pending.pop(0))
                    # dependency-free weight loads keep the PE activity monitor
                    # from re-throttling the clock during ACT-gated idles
                    for _ in range(FILL_LDW):
                        nc.tensor.ldweights(zero_t[:, :128])
                for item in pending:
                    emit_z(item)
                while unit_queue:
                    unit_queue.pop(0)(flush=True)
                if h == 0:
                    for bq in range(HALF):
                        nc.vector.tensor_copy(
                            zsb[:, bq * 512:(bq + 1) * 512],
                            zT[:, bq * 512:(bq + 1) * 512],
                        )

    nc.compile()
    return nc


def wlen_of(jhi, jlo, nch, lastw):
    return (jhi - jlo - 1) * 512 + (lastw if jhi == nch else 512)


def hs_of(h):
    return slice(h * 64, (h + 1) * 64)


def get_program(S=S_FULL):
    if S not in _prog_cache:
        _prog_cache[S] = _build_program(S)
    return _prog_cache[S]


def make_mask_band():
    """Triangle mask for the last 128 cols of a diagonal chunk:
    col t (relative to the diagonal start) is valid iff t <= r."""
    r = np.arange(128)[:, None]
    t = np.arange(128)[None, :]
    return np.where(t <= r, 0.0, NEG_BIG).astype(np.float32)


def make_core_inputs(x, W_K, W_Q, W_V, W_O, core):
    """Inputs for core = b*4 + g (batch b, head pair a0=2g, a1=2g+1)."""
    b, g = divmod(core, 4)
    a0, a1 = 2 * g, 2 * g + 1
    pdt = np.float16 if PROJ_DT == "fp16" else np.float32
    xT = np.ascontiguousarray(x[b].T).astype(pdt)
    wk = np.ascontiguousarray(np.concatenate([W_K[a0].T, W_K[a1].T], axis=1)).astype(pdt)
    wq = np.ascontiguousarray(np.concatenate([W_Q[a0].T, W_Q[a1].T], axis=1)).astype(pdt)
    wv = np.ascontiguousarray(np.concatenate([W_V[a0].T, W_V[a1].T], axis=1)).astype(pdt)
    wo0 = np.ascontiguousarray(np.concatenate([W_O[a0].T, W_O[a0].T], axis=0)).astype(pdt)
    wo1 = np.ascontiguousarray(np.concatenate([W_O[a1].T, W_O[a1].T], axis=0)).astype(pdt)
    tri = make_mask_band()
    return {
        "xT": xT, "wk": wk, "wq": wq, "wv": wv,
        "wo0": wo0, "wo1": wo1, "maskb": tri,
        "maskm": (tri == 0.0).astype(np.float16),
        "ident": np.eye(128, dtype=pdt),
    }


def run_on_cores(inputs, trace=False):
    from concourse.bass_utils import run_bass_kernel_spmd

    nc = get_program()
    in_maps = [
        make_core_inputs(
            inputs["x"], inputs["W_K"], inputs["W_Q"], inputs["W_V"],
            inputs["W_O"], core,
        )
        for core in range(N_CORES)
    ]
    return run_bass_kernel_spmd(
        nc, in_maps, list(range(N_CORES)), trace=trace,
    )


def kernel(x, W_K, W_Q, W_V, W_O):
    x = np.asarray(x, dtype=np.float32)
    W_K = np.asarray(W_K, dtype=np.float32)
    W_Q = np.asarray(W_Q, dtype=np.float32)
    W_V = np.asarray(W_V, dtype=np.float32)
    W_O = np.asarray(W_O, dtype=np.float32)
    res = run_on_cores(
        {"x": x, "W_K": W_K, "W_Q": W_Q, "W_V": W_V, "W_O": W_O}
    )
    out = np.zeros((B, S_FULL, E), dtype=np.float32)
    for b in range(B):
        acc = np.zeros((E, S_FULL), dtype=np.float32)
        for g in range(4):
            acc += res.results[b * 4 + g]["outT"].astype(np.float32)
        out[b] = acc.T
    return out
